# revision 57
# baseline (speedup 1.0000x reference)
"""Trainium2 Bass kernel for nn_Attention_21878563405851.

Module: kv = x1 @ W_qk (k,v split); q = x2 @ W_v; 8-head attention
(dim_head=64); out @ W_out + b_out.  B=2, N=2048, DIM=512.

Sharding over 8 NeuronCores: core c -> batch b=c//4, head pair
g=c%4 (heads 2g, 2g+1), ALL 2048 queries.  Tensor-parallel over
heads: every projection (q, k, v, out) is computed exactly once
system-wide -- no replication.  W_out is row-sharded; each core
emits a partial y^T and the 4-way reduction per batch happens on
the host during unshard (cheaper than this fabric's collectives).

Per core:
  1. DMA: x1/x2 as few large-run transfers (16KB contiguous per
     partition; 4KB-run chunked loads measured ~60-80 GB/s/queue vs
     ~200+ for big runs and dominated the original 27us lead-in).
     The four weight mats are packed into ONE [128, 4KB-row] tensor
     (separate [128, 1KB-row] tensors DMA'd as 128 tiny packets).
     x1 + x2(c0,c3) on the sync HW queue, weights + x2(c1) on the
     scalar HW queue, x2(c2) on the SW gpsimd queue.
  2. While inputs stream (~6.5-16us): PE pstate warm-up matmuls on
     a memset scratch tile (matmuls after idle run at ~1.2GHz for
     several us; warm they run 2.4GHz = 213ns/512 cols), plus a
     dummy exp to preload the ACT Exp table (1.28us, otherwise
     spent on the first real exp).
  3. Opener at x1-arrival: k proj (both halves) + first v tiles
     fill the PE window until x2 completes; then q proj qc0 and the
     8 attention blocks (qc, h): dots^T[kt] = k_h @ q_h^T -> exp
     (ACT, [128,1024] pair tiles, scale folded) -> attnv into
     [65, 512] PSUM (row 64 = denominator via a ones column in v).
     All other PE work (q qc1-3, key-major v proj, attnv pairs,
     out proj) drains from a global deadline FIFO behind the
     dots/exp stream, so the in-order PE queue always has ready
     work while ACT runs the 1.1us exps (ACT is the mid-stream
     pacer: 64 exps = 71us busy, achieved ~8us of gaps).
  4. normalization: denominator row -> SBUF, reciprocal_approx_fast
     (cannot read PSUM on hw; requires f32 in/out), bf16, then
     partition-broadcast via a PE rank-1 matmul (ones[1,64] x
     r[1,512] -> PSUM, ~0.2us vs 1.9us on GpSimd -- this chain is
     tail-critical), DVE multiply into o^T (bf16).
  5. out proj per qc: yp [128 d, 512 q] PSUM -> y_sb bf16; output
     DMA'd per qc (contiguous 4KB runs), last qc split per-dg so
     the final piece is small.

PSUM (8 banks): "big" [128,1024]x2 for k-proj halves + dots pairs;
"s5" [128,512]x2 for q/v proj, warm-up, broadcast tiles and
out-proj partials; "acc" [128,512]x2 for the attnv accumulators.

Tail (after the last exp): dummy warm matmuls woven into the flush
keep the PE at full pstate across the cross-engine waits; the last
block's broadcast tile uses a freed dt slot and its four output
drains run on the then-idle ACT engine, so the DVE only carries the
reciprocal chain + final multiply.

Measured: ~119us hw on a cool device (baseline batch x query-chunk
kernel: ~136us; first head-pair version before DMA/pstate/
scheduling/tail work: ~123us).  Run-to-run variance ~+-1us; a
heat-soaked device throttles all engine clocks ~20% and reads
~142us (90s idle restores).  fp8 DoubleRow was re-benched in
isolation (bench_dr.py): 427ns per 512-col matmul vs 216ns bf16 on
this silicon -- a 2x SLOWDOWN, so everything stays bf16 (fp8 q/k
would also cost 1.35% rel err vs the 2e-2 gate).
"""

import sys
from collections import deque

for _p in ("/opt/trn_rl_repo", "/root/.axon_site/_ro/trn_rl_repo"):
    if _p not in sys.path:
        sys.path.insert(0, _p)

import numpy as np
import ml_dtypes

import concourse.mybir as mybir
from concourse import tile
from concourse.bacc import Bacc

B, N, DIM = 2, 2048, 512
HEADS, DH = 8, 64
INNER = HEADS * DH
SCALE = DH ** -0.5
NCORES = 8
NKT = N // 128     # 16 key tiles
NC = DIM // 128    # 4 contraction chunks

BF16 = mybir.dt.bfloat16
F32 = mybir.dt.float32


def build_program():
    nc = Bacc(None, num_devices=NCORES)

    # ---- external I/O (per core), host-prearranged SBUF images ----
    x1T = nc.dram_tensor("x1T", [128, NC * N], BF16, kind="ExternalInput")
    x2T = nc.dram_tensor("x2T", [128, NC * N], BF16, kind="ExternalInput")
    # all four weight mats packed into one tensor: a [128, 1KB-row]
    # tensor DMAs as 128 tiny packets; one [128, 4KB-row] is 4x fewer
    wall = nc.dram_tensor("wall", [128, 4 * 512], BF16, kind="ExternalInput")
    # [p, (qc dg q)] bf16 partial output
    yT = nc.dram_tensor("yT", [128, 4 * N], BF16, kind="ExternalOutput")

    with tile.TileContext(nc) as tc:
        with (
            tc.tile_pool(name="xin", bufs=1) as xin,
            tc.tile_pool(name="wts", bufs=1) as wts,
            tc.tile_pool(name="kq", bufs=1) as kqp,
            tc.tile_pool(name="vex", bufs=1) as vexp,
            tc.tile_pool(name="et", bufs=20) as etp,
            tc.tile_pool(name="os", bufs=1) as osp,
            tc.tile_pool(name="ysb", bufs=1) as ysbp,
            tc.tile_pool(name="nrm", bufs=2) as nrmp,
            tc.tile_pool(name="ps", bufs=1, space="PSUM") as psp,
        ):
            # ---- load inputs: one big transfer per HW queue ----
            x1T_s = xin.tile([128, NC * N], BF16, name="x1T_s")
            x2T_s = xin.tile([128, NC * N], BF16, name="x2T_s")
            wall_s = wts.tile([128, 4 * 512], BF16, name="wall_s")
            wk_s = wall_s[:, 0:512]
            wq_s = wall_s[:, 512:1024]
            wv_s = wall_s[:, 1024:1536]
            wo_s = wall_s[:, 1536:2048]

            # x1 alone on the sync HW queue (arrives ~16us, gates k proj);
            # weights first on the scalar HW queue (~14.5us); x2 spread
            # over all three queues so q proj can start ~21us.
            nc.sync.dma_start(x1T_s[:], x1T[:])
            nc.sync.dma_start(x2T_s[:, 0:N], x2T[:, 0:N])
            nc.sync.dma_start(x2T_s[:, 3 * N:4 * N], x2T[:, 3 * N:4 * N])
            nc.scalar.dma_start(wall_s[:], wall[:])
            nc.scalar.dma_start(x2T_s[:, N:2 * N], x2T[:, N:2 * N])
            nc.gpsimd.dma_start(x2T_s[:, 2 * N:3 * N], x2T[:, 2 * N:3 * N])

            qT_s = kqp.tile([128, N], BF16, name="qT_s")
            kT_s = kqp.tile([128, N], BF16, name="kT_s")
            # v extended: per key tile, per head: 64 v cols + 1 ones col
            vE_s = vexp.tile([128, NKT, 2, 65], BF16, name="vE_s")
            nc.vector.memset(vE_s[:, :, :, 64:65], 1.0)
            ones_s = vexp.tile([1, 64], BF16, name="ones_s")
            nc.vector.memset(ones_s[:], 1.0)

            o_s = osp.tile([128, N], BF16, name="o_s")
            y_sb = ysbp.tile([128, 4, 4, 512], BF16, name="y_sb")

            def q_proj(t):
                qp = psp.tile([128, 512], F32, name=f"qp{t}", tag="s5", bufs=2)
                for c in range(NC):
                    nc.tensor.matmul(
                        qp[:],
                        wq_s[:, c * 128:(c + 1) * 128],
                        x2T_s[:, c * N + t * 512: c * N + (t + 1) * 512],
                        start=(c == 0),
                        stop=(c == NC - 1),
                    )
                nc.vector.tensor_copy(qT_s[:, t * 512:(t + 1) * 512], qp[:])

            kh_box = {}

            def k_part(half, cs):
                # one accumulation group split into two emission parts so
                # the FIFO can interleave them; cs = (0, 1) or (2, 3)
                if half not in kh_box:
                    kh_box[half] = psp.tile([128, 1024], F32,
                                            name=f"kh{half}", tag="big",
                                            bufs=2)
                kh = kh_box[half]
                for c in cs:
                    for j in range(2):
                        col = half * 1024 + j * 512
                        nc.tensor.matmul(
                            kh[:, j * 512:(j + 1) * 512],
                            wk_s[:, c * 128:(c + 1) * 128],
                            x1T_s[:, c * N + col: c * N + col + 512],
                            start=(c == 0),
                            stop=(c == NC - 1),
                        )
                if cs[-1] == NC - 1:
                    # split drain so the first dots only waits on 512 cols
                    for j in range(2):
                        nc.vector.tensor_copy(
                            kT_s[:, half * 1024 + j * 512:
                                 half * 1024 + (j + 1) * 512],
                            kh[:, j * 512:(j + 1) * 512],
                        )

            def k_half(half):
                k_part(half, (0, 1))
                k_part(half, (2, 3))

            def v_one(kt):
                # key-major v for one key tile (both heads + ones col)
                vp = psp.tile([128, 128], F32, name="vp", tag="s5", bufs=2)
                for c in range(NC):
                    nc.tensor.matmul(
                        vp[:],
                        x1T_s[:, c * N + kt * 128: c * N + (kt + 1) * 128],
                        wv_s[:, c * 128:(c + 1) * 128],
                        start=(c == 0),
                        stop=(c == NC - 1),
                    )
                nc.vector.tensor_copy(
                    vE_s[:, kt, :, 0:64],
                    vp.rearrange("p (h d) -> p h d", h=2),
                )

            # ---- PE pstate warm-up: dummy matmuls on a memset scratch
            # tile while x1/x2 stream in, so the real projections run at
            # full clock (matmuls after idle start at the 1.2GHz pstate).
            # Gated only on the memset, so they start at ~6.5us. ----
            scr_s = wts.tile([128, 512], BF16, name="scr_s")
            nc.vector.memset(scr_s[:], 0.0)
            # preload the ACT Exp table now (~1.3us) instead of lazily on
            # the first real exp, which sits on the critical path
            ew_s = wts.tile([1, 16], BF16, name="ew_s")
            nc.scalar.activation(ew_s[:], scr_s[0:1, 0:16],
                                 mybir.ActivationFunctionType.Exp)
            warm = psp.tile([128, 512], F32, name="warm", tag="s5", bufs=2)
            for _ in range(46):
                nc.tensor.matmul(warm[:], scr_s[:, 0:128], scr_s[:])

            # opener: k proj + early v tiles fill the PE window while x2
            # is still streaming (x1 lands ~6us before x2's last chunk)
            k_half(0)
            k_half(1)
            v_early = 6
            for kt in range(v_early):
                v_one(kt)
            q_proj(0)

            # ---- global deferred-PE-work FIFO ----
            fifo = deque()
            emitted = set()

            def run(e):
                e[2]()
                emitted.add(e[3])

            for t in (1, 2, 3):
                fifo.append((0, 1, lambda t=t: q_proj(t), f"q{t}"))
            for kt in range(v_early, NKT):
                fifo.append((0, 1, lambda kt=kt: v_one(kt), f"v{kt}"))

            # ---- attention: 8 blocks of (qc, h) ----
            ET_BUFS = 20

            for blk in range(8):
                qc, h = blk // 2, blk % 2
                r0 = h * 64
                acc = psp.tile([128, 512], F32, name=f"acc{blk}", tag="acc",
                               bufs=2)

                def emit_attnv(kp, e_t, acc=acc, h=h):
                    for j in range(2):
                        kt = 2 * kp + j
                        nc.tensor.matmul(
                            acc[0:65, :],
                            vE_s[:, kt, h, :],
                            e_t[:, j * 512:(j + 1) * 512],
                            start=(kt == 0),
                            stop=(kt == NKT - 1),
                        )

                for kp in range(8):
                    step = blk * 8 + kp
                    dt = psp.tile([128, 1024], F32, name="dt", tag="big",
                                  bufs=2)
                    for j in range(2):
                        kt = 2 * kp + j
                        nc.tensor.matmul(
                            dt[:, j * 512:(j + 1) * 512],
                            kT_s[r0:r0 + 64, kt * 128:(kt + 1) * 128],
                            qT_s[r0:r0 + 64, qc * 512:(qc + 1) * 512],
                        )
                    e_t = etp.tile([128, 1024], BF16, name="e_t", tag="e")
                    nc.scalar.activation(
                        e_t[:], dt[:],
                        mybir.ActivationFunctionType.Exp, scale=SCALE,
                    )
                    # block0's attnv is deferred further so the early steps
                    # have room for the v-proj fillers without starving ACT
                    fifo.append((step + (6 if blk == 0 else 2), 1,
                                 lambda kp=kp, e_t=e_t, f=emit_attnv:
                                 f(kp, e_t),
                                 f"av{blk}_{kp}"))

                    # mandatory pops: e_t ring safety + norm-lag bound
                    while fifo and (
                        (step >= ET_BUFS - 2 and
                         f"av{(step - ET_BUFS + 2) // 8}_"
                         f"{(step - ET_BUFS + 2) % 8}" not in emitted)
                        or (kp == 0 and blk >= 2 and
                            f"mult{blk - 2}" not in emitted)
                    ):
                        run(fifo.popleft())
                    # budgeted pops (~2 matmul-pairs of PE work per step);
                    # none on the block's last step so the next block's
                    # dots issue immediately.  Skip-ahead scan: a
                    # not-yet-eligible head (e.g. a y entry at last+9)
                    # must not block attnv entries behind it -- that
                    # head-blocking bunched all the boundary pops and
                    # caused ~0.9us ACT gaps.  Cross-family entries touch
                    # disjoint tiles, and relative order within each
                    # family (avN_*, normN/multN/y chains) is preserved
                    # because same-family eligibilities are monotone.
                    budget = 0 if kp == 7 else 2
                    while fifo and budget > 0 and fifo[0][0] <= step:
                        e = fifo.popleft()
                        budget -= e[1]
                        run(e)

                # normalization chain; broadcast via PE rank-1 matmul
                rb_box = []

                def norm_run(acc=acc, rb_box=rb_box, blk=blk):
                    s_s = nrmp.tile([1, 512], F32, name="s_s", tag="s")
                    nc.vector.tensor_copy(s_s[:], acc[64:65, :])
                    r_s = nrmp.tile([1, 512], F32, name="r_s", tag="r")
                    nc.vector.reciprocal_approx_fast(r_s[:], s_s[:])
                    r16 = nrmp.tile([1, 512], BF16, name="r16", tag="r16")
                    nc.vector.tensor_copy(r16[:], r_s[:])
                    rb_s = nrmp.tile([64, 512], BF16, name="rb_s", tag="rb")
                    if blk == 7:
                        # tail-critical: PE rank-1 matmul broadcast
                        # (~0.2us vs 1.9us on GpSimd) into a freed dt
                        # slot, drained to SBUF (DVE can't read two PSUM
                        # operands in one tensor_tensor)
                        rb_ps = psp.tile([64, 512], F32, name="rb_ps",
                                         tag="big", bufs=2)
                        nc.tensor.matmul(rb_ps[:], ones_s[:], r16[:])
                        nc.vector.tensor_copy(rb_s[:], rb_ps[:])
                    else:
                        # mid-stream: latency is hidden, so use the idle
                        # GpSimd engine -- no PE pop between dots (these
                        # caused ~300-600ns ACT gaps at block bounds) and
                        # no rb drain on DVE
                        nc.gpsimd.partition_broadcast(rb_s[:], r16[:])
                    rb_box.append(rb_s)

                def emit_mult(qc=qc, r0=r0, acc=acc, rb_box=rb_box):
                    nc.vector.tensor_mul(
                        o_s[r0:r0 + 64, qc * 512:(qc + 1) * 512],
                        acc[0:64, :], rb_box[0][:],
                    )

                last = blk * 8 + 7
                fifo.append((last + 2, 0, norm_run, f"norm{blk}"))
                # mid-stream mult waits the 1.9us GpSimd broadcast; give
                # it an extra step so it can't block the DVE queue head
                fifo.append((last + (3 if blk == 7 else 4), 0, emit_mult,
                             f"mult{blk}"))

                if h == 1:
                    def emit_y_dg(qc, dg):
                        yp = psp.tile([128, 512], F32, name=f"yp{qc}{dg}",
                                      tag="s5", bufs=2)
                        nc.tensor.matmul(
                            yp[:],
                            wo_s[:, dg * 128:(dg + 1) * 128],
                            o_s[:, qc * 512:(qc + 1) * 512],
                        )
                        if qc == 3:
                            # tail: drain on the idle ACT engine; DVE is
                            # busy with the mult and would serialize all
                            # four drains behind it
                            nc.scalar.activation(
                                y_sb[:, qc, dg, :], yp[:],
                                mybir.ActivationFunctionType.Copy,
                            )
                        else:
                            nc.vector.tensor_copy(y_sb[:, qc, dg, :], yp[:])
                        if qc == 3:
                            # tail-critical: ship each dg as it drains
                            eng = (nc.sync, nc.gpsimd)[dg % 2]
                            eng.dma_start(
                                yT[:, (qc * 4 + dg) * 512:
                                   (qc * 4 + dg + 1) * 512],
                                y_sb[:, qc, dg, :],
                            )
                        elif dg == 3:
                            # one contiguous [128, 4KB-run] transfer.
                            # Always the sync queue: on gpsimd this
                            # trigger (waiting 4 DVE drains) would block
                            # the next block's broadcast behind it in the
                            # in-order queue -> late mult -> the popped
                            # attnv 2 blocks later stalls the PE head on
                            # its acc-slot WAR (the ~0.9us ACT gaps seen
                            # at blocks 3/5/7).
                            nc.sync.dma_start(
                                yT[:, qc * 4 * 512:(qc + 1) * 4 * 512],
                                y_sb[:, qc, :, :],
                            )
                    # after the mult (+4) but not so late that the
                    # ineligible head blocks the next block's attnv
                    # entries behind it (pops are strictly in-order; a
                    # skip-ahead pop policy was tried and silently broke
                    # correctness -- see docstring)
                    for dg in range(4):
                        fifo.append((last + 5 + dg, 1,
                                     lambda qc=qc, dg=dg: emit_y_dg(qc, dg),
                                     f"y{qc}_{dg}"))

            # flush remaining deferred work, weaving in dummy warm
            # matmuls: the PE idles on cross-engine waits here (last
            # attnvs wait the last exps, out-proj waits the norm chain)
            # and drops to the 1.2GHz pstate, slowing the tail-critical
            # final matmuls (585ns vs 371ns measured).  Dummy targets:
            # fresh s5 tiles while the attnvs flush (their slot
            # predecessors have no readers), then dt's "big" slots once
            # the last exps have read them -- never a slot a live tile
            # (rb_ps/yp) still occupies.
            normed = False
            while fifo:
                e = fifo.popleft()
                if not normed:
                    # pre-norm only: post-norm dummies would rotate into
                    # rb_ps's slot and stall head-of-line on the mult
                    dmy = psp.tile([128, 512], F32, name="dmy", tag="s5",
                                   bufs=2)
                    nc.tensor.matmul(dmy[:], scr_s[:, 0:128], scr_s[:])
                if e[3].startswith("norm"):
                    normed = True
                run(e)

    nc.finalize()
    return nc


_NC_CACHE = None


def _get_program():
    global _NC_CACHE
    if _NC_CACHE is None:
        _NC_CACHE = build_program()
    return _NC_CACHE


def make_in_maps(x1, x2, W_qk, W_v, W_out, b_out):
    bf = ml_dtypes.bfloat16
    x1 = np.asarray(x1, np.float32)
    x2 = np.asarray(x2, np.float32)
    W_qk = np.asarray(W_qk, np.float32)
    W_v = np.asarray(W_v, np.float32)
    W_out = np.asarray(W_out, np.float32)

    # [p, (c k)] images: X[b]^T with the 512-dim contraction split into
    # 4 chunks of 128 partitions
    def xT_img(X):
        return np.ascontiguousarray(
            X.reshape(N, NC, 128).transpose(2, 1, 0).reshape(128, NC * N)
        ).astype(bf)

    x1T_imgs = [xT_img(x1[b]) for b in range(B)]
    x2T_imgs = [xT_img(x2[b]) for b in range(B)]

    # weight images per head-pair g: [p, (c f)] = W[c*128+p, g*128+f]
    def w_img(W, g):
        return np.ascontiguousarray(
            W[:, g * 128:(g + 1) * 128]
            .reshape(NC, 128, 128).transpose(1, 0, 2).reshape(128, NC * 128)
        ).astype(bf)

    # wall = [wk | wq | wv | wo], each [128, 512]
    # wo: rows for this head pair, [p, (dg f)] = W_out[g*128+p, dg*128+f]
    wall_imgs = [
        np.ascontiguousarray(np.concatenate([
            w_img(W_qk[:, :INNER], g),
            w_img(W_v, g),
            w_img(W_qk[:, INNER:], g),
            W_out[g * 128:(g + 1) * 128, :].astype(bf),
        ], axis=1))
        for g in range(4)
    ]

    in_maps = []
    for c in range(NCORES):
        b, g = c // 4, c % 4
        in_maps.append(
            {
                "x1T": x1T_imgs[b],
                "x2T": x2T_imgs[b],
                "wall": wall_imgs[g],
            }
        )
    return in_maps


def assemble_output(results, b_out):
    y = np.zeros((B, N, DIM), np.float32)
    for c in range(NCORES):
        b = c // 4
        yTc = np.asarray(results[c]["yT"], np.float32)  # [128, (qc dg q)]
        # yTc[p, qc, dg, q] = y_part[qc*512+q, dg*128+p]
        D = yTc.reshape(128, 4, 4, 512)
        # -> [qc, q, dg, p] -> [N, DIM]
        y[b] += D.transpose(1, 3, 2, 0).reshape(N, DIM)
    y += np.asarray(b_out, np.float32)
    return y


def kernel(x1, x2, W_qk, W_v, W_out, b_out):
    from concourse.bass_utils import run_bass_kernel_spmd

    nc = _get_program()
    in_maps = make_in_maps(x1, x2, W_qk, W_v, W_out, b_out)
    res = run_bass_kernel_spmd(nc, in_maps, list(range(NCORES)))
    return assemble_output(res.results, b_out)


# revision 58
# speedup vs baseline: 1.0036x; 1.0036x over previous
"""Trainium2 Bass kernel for nn_Attention_21878563405851.

Module: kv = x1 @ W_qk (k,v split); q = x2 @ W_v; 8-head attention
(dim_head=64); out @ W_out + b_out.  B=2, N=2048, DIM=512.

Sharding over 8 NeuronCores: core c -> batch b=c//4, head pair
g=c%4 (heads 2g, 2g+1), ALL 2048 queries.  Tensor-parallel over
heads: every projection (q, k, v, out) is computed exactly once
system-wide -- no replication.  W_out is row-sharded; each core
emits a partial y^T and the 4-way reduction per batch happens on
the host during unshard (cheaper than this fabric's collectives).

Per core:
  1. DMA: x1/x2 as few large-run transfers (16KB contiguous per
     partition; 4KB-run chunked loads measured ~60-80 GB/s/queue vs
     ~200+ for big runs and dominated the original 27us lead-in).
     The four weight mats are packed into ONE [128, 4KB-row] tensor
     (separate [128, 1KB-row] tensors DMA'd as 128 tiny packets).
     x1 + x2(c0,c3) on the sync HW queue, weights + x2(c1) on the
     scalar HW queue, x2(c2) on the SW gpsimd queue.
  2. While inputs stream (~6.5-16us): PE pstate warm-up matmuls on
     a memset scratch tile (matmuls after idle run at ~1.2GHz for
     several us; warm they run 2.4GHz = 213ns/512 cols), plus a
     dummy exp to preload the ACT Exp table (1.28us, otherwise
     spent on the first real exp).
  3. Opener at x1-arrival: k proj (both halves) + first v tiles
     fill the PE window until x2 completes; then q proj qc0 and the
     8 attention blocks (qc, h): dots^T[kt] = k_h @ q_h^T -> exp
     (ACT, [128,1024] pair tiles, scale folded) -> attnv into
     [65, 512] PSUM (row 64 = denominator via a ones column in v).
     All other PE work (q qc1-3, key-major v proj, attnv pairs,
     out proj) drains from a global deadline FIFO behind the
     dots/exp stream, so the in-order PE queue always has ready
     work while ACT runs the 1.1us exps (ACT is the mid-stream
     pacer: 64 exps = 71us busy, achieved ~8us of gaps).
  4. normalization: denominator row -> SBUF, reciprocal_approx_fast
     (cannot read PSUM on hw; requires f32 in/out), bf16, then
     partition-broadcast via a PE rank-1 matmul (ones[1,64] x
     r[1,512] -> PSUM, ~0.2us vs 1.9us on GpSimd -- this chain is
     tail-critical), DVE multiply into o^T (bf16).
  5. out proj per qc: yp [128 d, 512 q] PSUM -> y_sb bf16; output
     DMA'd per qc (contiguous 4KB runs), last qc split per-dg so
     the final piece is small.

PSUM (8 banks): "big" [128,1024]x2 for k-proj halves + dots pairs;
"s5" [128,512]x2 for q/v proj, warm-up, broadcast tiles and
out-proj partials; "acc" [128,512]x2 for the attnv accumulators.

Scheduling invariant: the deferred-work FIFO must emit strictly in
order.  A skip-ahead pop policy (emitting eligible entries past an
ineligible head) is safe by data-dependency analysis of the tiles
involved, yet silently corrupts results (rel err 0.32) -- the Tile
framework has a hidden emission-order dependency (suspect: PSUM
accumulation-group interleaving or buffer-rotation WAR tracking).
Consequence: ~0.9us ACT gaps at the three out-proj block boundaries
(~2.7us total) are conserved under every eligibility setting and
remain unfixed.

Tail (after the last exp): dummy warm matmuls woven into the flush
keep the PE at full pstate across the cross-engine waits; the last
block's broadcast tile uses a freed dt slot and its four output
drains run on the then-idle ACT engine, so the DVE only carries the
reciprocal chain + final multiply.

Measured: ~119us hw on a cool device (baseline batch x query-chunk
kernel: ~136us; first head-pair version before DMA/pstate/
scheduling/tail work: ~123us).  Run-to-run variance ~+-1us; a
heat-soaked device throttles all engine clocks ~20% and reads
~142us (90s idle restores).  fp8 DoubleRow was re-benched in
isolation (bench_dr.py): 427ns per 512-col matmul vs 216ns bf16 on
this silicon -- a 2x SLOWDOWN, so everything stays bf16 (fp8 q/k
would also cost 1.35% rel err vs the 2e-2 gate).
"""

import sys
from collections import deque

for _p in ("/opt/trn_rl_repo", "/root/.axon_site/_ro/trn_rl_repo"):
    if _p not in sys.path:
        sys.path.insert(0, _p)

import numpy as np
import ml_dtypes

import concourse.mybir as mybir
from concourse import tile
from concourse.bacc import Bacc

B, N, DIM = 2, 2048, 512
HEADS, DH = 8, 64
INNER = HEADS * DH
SCALE = DH ** -0.5
NCORES = 8
NKT = N // 128     # 16 key tiles
NC = DIM // 128    # 4 contraction chunks

BF16 = mybir.dt.bfloat16
F32 = mybir.dt.float32


def build_program():
    nc = Bacc(None, num_devices=NCORES)

    # ---- external I/O (per core), host-prearranged SBUF images ----
    x1T = nc.dram_tensor("x1T", [128, NC * N], BF16, kind="ExternalInput")
    x2T = nc.dram_tensor("x2T", [128, NC * N], BF16, kind="ExternalInput")
    # all four weight mats packed into one tensor: a [128, 1KB-row]
    # tensor DMAs as 128 tiny packets; one [128, 4KB-row] is 4x fewer
    wall = nc.dram_tensor("wall", [128, 4 * 512], BF16, kind="ExternalInput")
    # [p, (qc dg q)] bf16 partial output
    yT = nc.dram_tensor("yT", [128, 4 * N], BF16, kind="ExternalOutput")

    with tile.TileContext(nc) as tc:
        with (
            tc.tile_pool(name="xin", bufs=1) as xin,
            tc.tile_pool(name="wts", bufs=1) as wts,
            tc.tile_pool(name="kq", bufs=1) as kqp,
            tc.tile_pool(name="vex", bufs=1) as vexp,
            tc.tile_pool(name="et", bufs=20) as etp,
            tc.tile_pool(name="os", bufs=1) as osp,
            tc.tile_pool(name="ysb", bufs=1) as ysbp,
            tc.tile_pool(name="nrm", bufs=2) as nrmp,
            tc.tile_pool(name="ps", bufs=1, space="PSUM") as psp,
        ):
            # ---- load inputs: one big transfer per HW queue ----
            x1T_s = xin.tile([128, NC * N], BF16, name="x1T_s")
            x2T_s = xin.tile([128, NC * N], BF16, name="x2T_s")
            wall_s = wts.tile([128, 4 * 512], BF16, name="wall_s")
            wk_s = wall_s[:, 0:512]
            wq_s = wall_s[:, 512:1024]
            wv_s = wall_s[:, 1024:1536]
            wo_s = wall_s[:, 1536:2048]

            # x1 alone on the sync HW queue (arrives ~16us, gates k proj);
            # weights first on the scalar HW queue (~14.5us); x2 spread
            # over all three queues so q proj can start ~21us.
            nc.sync.dma_start(x1T_s[:], x1T[:])
            nc.sync.dma_start(x2T_s[:, 0:N], x2T[:, 0:N])
            nc.sync.dma_start(x2T_s[:, 3 * N:4 * N], x2T[:, 3 * N:4 * N])
            nc.scalar.dma_start(wall_s[:], wall[:])
            nc.scalar.dma_start(x2T_s[:, N:2 * N], x2T[:, N:2 * N])
            nc.gpsimd.dma_start(x2T_s[:, 2 * N:3 * N], x2T[:, 2 * N:3 * N])

            qT_s = kqp.tile([128, N], BF16, name="qT_s")
            kT_s = kqp.tile([128, N], BF16, name="kT_s")
            # v extended: per key tile, per head: 64 v cols + 1 ones col
            vE_s = vexp.tile([128, NKT, 2, 65], BF16, name="vE_s")
            nc.vector.memset(vE_s[:, :, :, 64:65], 1.0)
            ones_s = vexp.tile([1, 64], BF16, name="ones_s")
            nc.vector.memset(ones_s[:], 1.0)

            o_s = osp.tile([128, N], BF16, name="o_s")
            y_sb = ysbp.tile([128, 4, 4, 512], BF16, name="y_sb")

            def q_proj(t):
                qp = psp.tile([128, 512], F32, name=f"qp{t}", tag="s5", bufs=2)
                for c in range(NC):
                    nc.tensor.matmul(
                        qp[:],
                        wq_s[:, c * 128:(c + 1) * 128],
                        x2T_s[:, c * N + t * 512: c * N + (t + 1) * 512],
                        start=(c == 0),
                        stop=(c == NC - 1),
                    )
                nc.vector.tensor_copy(qT_s[:, t * 512:(t + 1) * 512], qp[:])

            kh_box = {}

            def k_part(half, cs):
                # one accumulation group split into two emission parts so
                # the FIFO can interleave them; cs = (0, 1) or (2, 3)
                if half not in kh_box:
                    kh_box[half] = psp.tile([128, 1024], F32,
                                            name=f"kh{half}", tag="big",
                                            bufs=2)
                kh = kh_box[half]
                for c in cs:
                    for j in range(2):
                        col = half * 1024 + j * 512
                        nc.tensor.matmul(
                            kh[:, j * 512:(j + 1) * 512],
                            wk_s[:, c * 128:(c + 1) * 128],
                            x1T_s[:, c * N + col: c * N + col + 512],
                            start=(c == 0),
                            stop=(c == NC - 1),
                        )
                if cs[-1] == NC - 1:
                    # split drain so the first dots only waits on 512 cols
                    for j in range(2):
                        nc.vector.tensor_copy(
                            kT_s[:, half * 1024 + j * 512:
                                 half * 1024 + (j + 1) * 512],
                            kh[:, j * 512:(j + 1) * 512],
                        )

            def k_half(half):
                k_part(half, (0, 1))
                k_part(half, (2, 3))

            def v_one(kt):
                # key-major v for one key tile (both heads + ones col)
                vp = psp.tile([128, 128], F32, name="vp", tag="s5", bufs=2)
                for c in range(NC):
                    nc.tensor.matmul(
                        vp[:],
                        x1T_s[:, c * N + kt * 128: c * N + (kt + 1) * 128],
                        wv_s[:, c * 128:(c + 1) * 128],
                        start=(c == 0),
                        stop=(c == NC - 1),
                    )
                nc.vector.tensor_copy(
                    vE_s[:, kt, :, 0:64],
                    vp.rearrange("p (h d) -> p h d", h=2),
                )

            # ---- PE pstate warm-up: dummy matmuls on a memset scratch
            # tile while x1/x2 stream in, so the real projections run at
            # full clock (matmuls after idle start at the 1.2GHz pstate).
            # Gated only on the memset, so they start at ~6.5us. ----
            scr_s = wts.tile([128, 512], BF16, name="scr_s")
            nc.vector.memset(scr_s[:], 0.0)
            # preload the ACT Exp table now (~1.3us) instead of lazily on
            # the first real exp, which sits on the critical path
            ew_s = wts.tile([1, 16], BF16, name="ew_s")
            nc.scalar.activation(ew_s[:], scr_s[0:1, 0:16],
                                 mybir.ActivationFunctionType.Exp)
            warm = psp.tile([128, 512], F32, name="warm", tag="s5", bufs=2)
            for _ in range(46):
                nc.tensor.matmul(warm[:], scr_s[:, 0:128], scr_s[:])

            # opener: k proj + early v tiles fill the PE window while x2
            # is still streaming (x1 lands ~6us before x2's last chunk)
            k_half(0)
            k_half(1)
            v_early = 6
            for kt in range(v_early):
                v_one(kt)
            q_proj(0)

            # ---- global deferred-PE-work FIFO ----
            fifo = deque()
            emitted = set()

            def run(e):
                e[2]()
                emitted.add(e[3])

            for t in (1, 2, 3):
                fifo.append((0, 1, lambda t=t: q_proj(t), f"q{t}"))
            for kt in range(v_early, NKT):
                fifo.append((0, 1, lambda kt=kt: v_one(kt), f"v{kt}"))

            # ---- attention: 8 blocks of (qc, h) ----
            ET_BUFS = 20

            for blk in range(8):
                qc, h = blk // 2, blk % 2
                r0 = h * 64
                acc = psp.tile([128, 512], F32, name=f"acc{blk}", tag="acc",
                               bufs=2)

                def emit_attnv(kp, e_t, acc=acc, h=h):
                    for j in range(2):
                        kt = 2 * kp + j
                        nc.tensor.matmul(
                            acc[0:65, :],
                            vE_s[:, kt, h, :],
                            e_t[:, j * 512:(j + 1) * 512],
                            start=(kt == 0),
                            stop=(kt == NKT - 1),
                        )

                for kp in range(8):
                    step = blk * 8 + kp
                    dt = psp.tile([128, 1024], F32, name="dt", tag="big",
                                  bufs=2)
                    for j in range(2):
                        kt = 2 * kp + j
                        nc.tensor.matmul(
                            dt[:, j * 512:(j + 1) * 512],
                            kT_s[r0:r0 + 64, kt * 128:(kt + 1) * 128],
                            qT_s[r0:r0 + 64, qc * 512:(qc + 1) * 512],
                        )
                    e_t = etp.tile([128, 1024], BF16, name="e_t", tag="e")
                    nc.scalar.activation(
                        e_t[:], dt[:],
                        mybir.ActivationFunctionType.Exp, scale=SCALE,
                    )
                    # block0's attnv is deferred further so the early steps
                    # have room for the v-proj fillers without starving ACT
                    fifo.append((step + (6 if blk == 0 else 2), 1,
                                 lambda kp=kp, e_t=e_t, f=emit_attnv:
                                 f(kp, e_t),
                                 f"av{blk}_{kp}"))

                    # mandatory pops: e_t ring safety + norm-lag bound
                    while fifo and (
                        (step >= ET_BUFS - 2 and
                         f"av{(step - ET_BUFS + 2) // 8}_"
                         f"{(step - ET_BUFS + 2) % 8}" not in emitted)
                        or (kp == 0 and blk >= 2 and
                            f"mult{blk - 2}" not in emitted)
                    ):
                        run(fifo.popleft())
                    # budgeted pops (~2 matmul-pairs of PE work per step);
                    # none on the block's last step so the next block's
                    # dots issue immediately.  Skip-ahead scan: a
                    # not-yet-eligible head (e.g. a y entry at last+9)
                    # must not block attnv entries behind it -- that
                    # head-blocking bunched all the boundary pops and
                    # caused ~0.9us ACT gaps.  Cross-family entries touch
                    # disjoint tiles, and relative order within each
                    # family (avN_*, normN/multN/y chains) is preserved
                    # because same-family eligibilities are monotone.
                    budget = 0 if kp == 7 else 2
                    while fifo and budget > 0 and fifo[0][0] <= step:
                        e = fifo.popleft()
                        budget -= e[1]
                        run(e)

                # normalization chain; broadcast via PE rank-1 matmul
                rb_box = []

                def norm_run(acc=acc, rb_box=rb_box, blk=blk):
                    s_s = nrmp.tile([1, 512], F32, name="s_s", tag="s")
                    nc.vector.tensor_copy(s_s[:], acc[64:65, :])
                    r_s = nrmp.tile([1, 512], F32, name="r_s", tag="r")
                    nc.vector.reciprocal_approx_fast(r_s[:], s_s[:])
                    r16 = nrmp.tile([1, 512], BF16, name="r16", tag="r16")
                    nc.vector.tensor_copy(r16[:], r_s[:])
                    rb_s = nrmp.tile([64, 512], BF16, name="rb_s", tag="rb")
                    if blk == 7:
                        # tail-critical: PE rank-1 matmul broadcast
                        # (~0.2us vs 1.9us on GpSimd) into a freed dt
                        # slot, drained to SBUF (DVE can't read two PSUM
                        # operands in one tensor_tensor)
                        rb_ps = psp.tile([64, 512], F32, name="rb_ps",
                                         tag="big", bufs=2)
                        nc.tensor.matmul(rb_ps[:], ones_s[:], r16[:])
                        nc.vector.tensor_copy(rb_s[:], rb_ps[:])
                    else:
                        # mid-stream: latency is hidden, so use the idle
                        # GpSimd engine -- no PE pop between dots (these
                        # caused ~300-600ns ACT gaps at block bounds) and
                        # no rb drain on DVE
                        nc.gpsimd.partition_broadcast(rb_s[:], r16[:])
                    rb_box.append(rb_s)

                def emit_mult(qc=qc, r0=r0, acc=acc, rb_box=rb_box):
                    nc.vector.tensor_mul(
                        o_s[r0:r0 + 64, qc * 512:(qc + 1) * 512],
                        acc[0:64, :], rb_box[0][:],
                    )

                last = blk * 8 + 7
                fifo.append((last + 2, 0, norm_run, f"norm{blk}"))
                # mid-stream mult waits the 1.9us GpSimd broadcast; give
                # it an extra step so it can't block the DVE queue head
                fifo.append((last + (3 if blk == 7 else 4), 0, emit_mult,
                             f"mult{blk}"))

                if h == 1:
                    def emit_y_dg(qc, dg):
                        yp = psp.tile([128, 512], F32, name=f"yp{qc}{dg}",
                                      tag="s5", bufs=2)
                        nc.tensor.matmul(
                            yp[:],
                            wo_s[:, dg * 128:(dg + 1) * 128],
                            o_s[:, qc * 512:(qc + 1) * 512],
                        )
                        if qc == 3:
                            # tail: drain on the idle ACT engine; DVE is
                            # busy with the mult and would serialize all
                            # four drains behind it
                            nc.scalar.activation(
                                y_sb[:, qc, dg, :], yp[:],
                                mybir.ActivationFunctionType.Copy,
                            )
                        else:
                            nc.vector.tensor_copy(y_sb[:, qc, dg, :], yp[:])
                        if qc == 3:
                            # tail-critical: ship each dg as it drains
                            eng = (nc.sync, nc.gpsimd)[dg % 2]
                            eng.dma_start(
                                yT[:, (qc * 4 + dg) * 512:
                                   (qc * 4 + dg + 1) * 512],
                                y_sb[:, qc, dg, :],
                            )
                        elif dg == 3:
                            # one contiguous [128, 4KB-run] transfer.
                            # Always the sync queue: on gpsimd this
                            # trigger (waiting 4 DVE drains) would block
                            # the next block's broadcast behind it in the
                            # in-order queue -> late mult -> the popped
                            # attnv 2 blocks later stalls the PE head on
                            # its acc-slot WAR (the ~0.9us ACT gaps seen
                            # at blocks 3/5/7).
                            nc.sync.dma_start(
                                yT[:, qc * 4 * 512:(qc + 1) * 4 * 512],
                                y_sb[:, qc, :, :],
                            )
                    # after the mult (+4) but not so late that the
                    # ineligible head blocks the next block's attnv
                    # entries behind it (pops are strictly in-order; a
                    # skip-ahead pop policy was tried and silently broke
                    # correctness -- see docstring)
                    for dg in range(4):
                        fifo.append((last + 5 + dg, 1,
                                     lambda qc=qc, dg=dg: emit_y_dg(qc, dg),
                                     f"y{qc}_{dg}"))

            # flush remaining deferred work, weaving in dummy warm
            # matmuls: the PE idles on cross-engine waits here (last
            # attnvs wait the last exps, out-proj waits the norm chain)
            # and drops to the 1.2GHz pstate, slowing the tail-critical
            # final matmuls (585ns vs 371ns measured).  Dummy targets:
            # fresh s5 tiles while the attnvs flush (their slot
            # predecessors have no readers), then dt's "big" slots once
            # the last exps have read them -- never a slot a live tile
            # (rb_ps/yp) still occupies.
            normed = False
            while fifo:
                e = fifo.popleft()
                if not normed:
                    # pre-norm only: post-norm dummies would rotate into
                    # rb_ps's slot and stall head-of-line on the mult
                    dmy = psp.tile([128, 512], F32, name="dmy", tag="s5",
                                   bufs=2)
                    nc.tensor.matmul(dmy[:], scr_s[:, 0:128], scr_s[:])
                if e[3].startswith("norm"):
                    normed = True
                run(e)

    nc.finalize()
    return nc


_NC_CACHE = None


def _get_program():
    global _NC_CACHE
    if _NC_CACHE is None:
        _NC_CACHE = build_program()
    return _NC_CACHE


def make_in_maps(x1, x2, W_qk, W_v, W_out, b_out):
    bf = ml_dtypes.bfloat16
    x1 = np.asarray(x1, np.float32)
    x2 = np.asarray(x2, np.float32)
    W_qk = np.asarray(W_qk, np.float32)
    W_v = np.asarray(W_v, np.float32)
    W_out = np.asarray(W_out, np.float32)

    # [p, (c k)] images: X[b]^T with the 512-dim contraction split into
    # 4 chunks of 128 partitions
    def xT_img(X):
        return np.ascontiguousarray(
            X.reshape(N, NC, 128).transpose(2, 1, 0).reshape(128, NC * N)
        ).astype(bf)

    x1T_imgs = [xT_img(x1[b]) for b in range(B)]
    x2T_imgs = [xT_img(x2[b]) for b in range(B)]

    # weight images per head-pair g: [p, (c f)] = W[c*128+p, g*128+f]
    def w_img(W, g):
        return np.ascontiguousarray(
            W[:, g * 128:(g + 1) * 128]
            .reshape(NC, 128, 128).transpose(1, 0, 2).reshape(128, NC * 128)
        ).astype(bf)

    # wall = [wk | wq | wv | wo], each [128, 512]
    # wo: rows for this head pair, [p, (dg f)] = W_out[g*128+p, dg*128+f]
    wall_imgs = [
        np.ascontiguousarray(np.concatenate([
            w_img(W_qk[:, :INNER], g),
            w_img(W_v, g),
            w_img(W_qk[:, INNER:], g),
            W_out[g * 128:(g + 1) * 128, :].astype(bf),
        ], axis=1))
        for g in range(4)
    ]

    in_maps = []
    for c in range(NCORES):
        b, g = c // 4, c % 4
        in_maps.append(
            {
                "x1T": x1T_imgs[b],
                "x2T": x2T_imgs[b],
                "wall": wall_imgs[g],
            }
        )
    return in_maps


def assemble_output(results, b_out):
    y = np.zeros((B, N, DIM), np.float32)
    for c in range(NCORES):
        b = c // 4
        yTc = np.asarray(results[c]["yT"], np.float32)  # [128, (qc dg q)]
        # yTc[p, qc, dg, q] = y_part[qc*512+q, dg*128+p]
        D = yTc.reshape(128, 4, 4, 512)
        # -> [qc, q, dg, p] -> [N, DIM]
        y[b] += D.transpose(1, 3, 2, 0).reshape(N, DIM)
    y += np.asarray(b_out, np.float32)
    return y


def kernel(x1, x2, W_qk, W_v, W_out, b_out):
    from concourse.bass_utils import run_bass_kernel_spmd

    nc = _get_program()
    in_maps = make_in_maps(x1, x2, W_qk, W_v, W_out, b_out)
    res = run_bass_kernel_spmd(nc, in_maps, list(range(NCORES)))
    return assemble_output(res.results, b_out)
